# revision 7
# baseline (speedup 1.0000x reference)
"""Causal self-attention (B=2, S=2048, E=1024, H=16, D=64) on 8 NeuronCores, v3.

Sharding: core = (batch b, head-group g of 4 heads); data parallel on B,
tensor parallel on heads.  Host sums the 4 partial output projections.

v3 vs v2 (126.2us -> target ~88us): q/k projections via full-array fp8
DoubleRow matmuls — host ships x8 [128, 4(b), 2(t), S] fp8 (e = 256b+128t+i)
and wq8/wk8 [128, 2(ab), 4(b), 2(t), 128] fp8 scaled by 64, so each
projection chain is 4 DR matmuls of 256-deep contraction at 0.5 cyc/col
(16.4k PE cyc total vs 65.5k bf16).  Scores psum is scaled 64^2; the exp
scale absorbs it (0.125/4096).  Numerics sim: rel err ~1.5e-2 < 2e-2 gate.

Layouts per core:
  xT_sb [128, 8ec, 2048]  bf16   x[b]^T  (for the v projection, bf16-clean)
  x8_sb [128, 4, 2, 2048] fp8    DR-interleaved x for q/k chains
  wq8/wk8 host-permuted per v2's chain A/B column split (chain ab column
    c = (h in 4, k in 32) -> d = 32*ab + k), fp8 * 64.
  qk chain (t, dst, ab): 4 DR matmuls (256-contraction blocks) -> psum
    [128, 512] = 64*q slice; DVE quantize copy -> q8/k8 [128, 2, S] fp8.
  scores: DR matmul per (head, kv-chunk): stat k8[32h:32h+32, :, 128c:+128],
    mov q8[32h:32h+32, :, 512t:+512] -> stp [128 sk, 512 sq] f32.
  exp on ACT per chunk-pair [128, 1024] f32->bf16, scale=2^-15 fused; no
    max-subtraction (scores provably bounded).  Diagonal-band pairs use a
    half-width exp; causal tri mask multiplied post-exp on DVE.
  attn@V: stat = v_sb[:, c, h, 0:65] (col 64 = ones -> rowsum), mov =
    ptile[:, ci, lo:512] -> pacc [65, 512] f32 accumulated over chunks in
    one PSUM bank; diagonal chunks use a partial moving width (lo = 128j).
  normalize: DVE recip of pacc[64:65, :], gpsimd partition_broadcast to 64
    rows, one DVE multiply writing attT[hp][64*hl:, cols] bf16 directly.
  oproj: per (slice, e-tile): 2 matmuls (hp) wo_sb x attT -> [128, 512] f32,
    copied bf16 and DMA'd out; host converts to f32 and reduces.
"""

import sys

sys.path.insert(0, "/opt/trn_rl_repo")

import numpy as np
import ml_dtypes

import concourse.bass as bass
import concourse.bacc as bacc
import concourse.mybir as mybir
import concourse.tile as tile
from concourse import bass_utils
from concourse import library_config

F32 = mybir.dt.float32
BF16 = mybir.dt.bfloat16
FP8 = mybir.dt.float8e4
AF = mybir.ActivationFunctionType
DR = mybir.MatmulPerfMode.DoubleRow
NBF = ml_dtypes.bfloat16
NF8 = ml_dtypes.float8_e4m3

B, S, E, H, D = 2, 2048, 1024, 16, 64
HPC = 4                 # heads per core
DP = HPC * D            # 256 d' columns per core
NCORES = 8
EC = E // 128           # 8 e-chunks
NSC = S // 128          # 16 kv chunks
NSL = 4                 # s-slices (512 wide); slice t <-> query block qb=t
SQ = 512                # query block width
NB = SQ // 128          # 4 sq sub-blocks per query block
WSCALE = 64.0           # fp8 weight upscale (q/k psum carries 64x)
EXPSCALE = 0.125 / (WSCALE * WSCALE)


def make_tri():
    # multiplicative causal mask for a diagonal 128x128 square of an
    # [sk, sq] tile: keep sq >= sk
    return (np.arange(128)[None, :] >= np.arange(128)[:, None]).astype(NBF)


def build_kernel(debug=False):
    nc = bacc.Bacc("TRN2", target_bir_lowering=False, debug=False)

    xT_d = nc.dram_tensor("xT", [E, S], BF16, kind="ExternalInput")
    x8_d = nc.dram_tensor("x8", [128, 4, 2, S], FP8, kind="ExternalInput")
    wq8_d = nc.dram_tensor("wq8", [128, 2, 4, 2, 128], FP8, kind="ExternalInput")
    wk8_d = nc.dram_tensor("wk8", [128, 2, 4, 2, 128], FP8, kind="ExternalInput")
    wv_d = nc.dram_tensor("wv", [E, DP], BF16, kind="ExternalInput")
    wo_d = nc.dram_tensor("wo", [DP, E], BF16, kind="ExternalInput")
    tri_d = nc.dram_tensor("tri", [128, 128], BF16, kind="ExternalInput")
    oT_d = nc.dram_tensor("oT", [E, S], BF16, kind="ExternalOutput")
    if debug:
        dbg = {
            "d_q8": nc.dram_tensor("d_q8", [128, 2, S], FP8, kind="ExternalOutput"),
            "d_k8": nc.dram_tensor("d_k8", [128, 2, S], FP8, kind="ExternalOutput"),
            "d_v": nc.dram_tensor("d_v", [128, NSC * HPC * 65], BF16, kind="ExternalOutput"),
            "d_attT0": nc.dram_tensor("d_attT0", [128, S], BF16, kind="ExternalOutput"),
            "d_attT1": nc.dram_tensor("d_attT1", [128, S], BF16, kind="ExternalOutput"),
        }

    xTr = xT_d.rearrange("(c p) s -> p c s", p=128)
    wvr = wv_d.rearrange("(c p) d -> p c d", p=128)
    wor = wo_d.rearrange("(c p) e -> p c e", p=128)
    oTr = oT_d.rearrange("(a p) s -> p a s", p=128)

    with tile.TileContext(nc) as tc:
        with tc.tile_pool(name="big", bufs=1) as big, \
             tc.tile_pool(name="pt", bufs=10) as pt_pool, \
             tc.tile_pool(name="rc", bufs=4) as rc_pool, \
             tc.tile_pool(name="rb", bufs=4) as rb_pool, \
             tc.tile_pool(name="og", bufs=4) as og_pool, \
             tc.tile_pool(name="st", bufs=2, space="PSUM") as st_pool, \
             tc.tile_pool(name="pa", bufs=2, space="PSUM") as pa_pool, \
             tc.tile_pool(name="gm", bufs=2, space="PSUM") as gm_pool:

            xT_sb = big.tile([128, EC, S], BF16, tag="xT", name="xT_sb")
            x8_sb = big.tile([128, 4, 2, S], FP8, tag="x8", name="x8_sb")
            wq8_sb = big.tile([128, 2, 4, 2, 128], FP8, tag="wq8", name="wq8_sb")
            wk8_sb = big.tile([128, 2, 4, 2, 128], FP8, tag="wk8", name="wk8_sb")
            wv_sb = big.tile([128, EC, DP], BF16, tag="wv", name="wv_sb")
            wo_sb = big.tile([128, 2, E], BF16, tag="wo", name="wo_sb")
            q8 = big.tile([128, 2, S], FP8, tag="q8", name="q8")
            k8 = big.tile([128, 2, S], FP8, tag="k8", name="k8")
            v_sb = big.tile([128, NSC, HPC, 65], BF16, tag="v", name="v_sb")
            attT = [big.tile([128, S], BF16, tag=f"attT{i}", name=f"attT{i}")
                    for i in range(2)]
            tri_sb = big.tile([128, 128], BF16, tag="tri", name="tri_sb")
            wmt = big.tile([1, 448], BF16, tag="wmt", name="wmt")
            warm = big.tile([1, 1], F32, tag="warm", name="warm")

            # ---- preamble: DMAs first (sync ring is serial — order by
            # first-consumer time), then gpsimd library + exp/PE warm ----
            nc.sync.dma_start(x8_sb[:, :, :, 0:512], x8_d[:, :, :, 0:512])
            nc.sync.dma_start(wq8_sb[:], wq8_d[:, :, :, :, :])
            nc.sync.dma_start(wk8_sb[:], wk8_d[:, :, :, :, :])
            nc.sync.dma_start(tri_sb[:], tri_d[:, :])
            nc.sync.dma_start(xT_sb[:, :, 0:256], xTr[:, :, 0:256])
            nc.sync.dma_start(wv_sb[:], wvr[:, :, :])
            nc.sync.dma_start(xT_sb[:, :, 256:512], xTr[:, :, 256:512])
            nc.sync.dma_start(x8_sb[:, :, :, 512:1024], x8_d[:, :, :, 512:1024])
            nc.sync.dma_start(xT_sb[:, :, 512:1024], xTr[:, :, 512:1024])
            nc.sync.dma_start(x8_sb[:, :, :, 1024:2048], x8_d[:, :, :, 1024:2048])
            nc.sync.dma_start(wo_sb[:], wor[:, :, :])
            nc.sync.dma_start(xT_sb[:, :, 1024:1536], xTr[:, :, 1024:1536])
            nc.sync.dma_start(xT_sb[:, :, 1536:2048], xTr[:, :, 1536:2048])
            nc.gpsimd.load_library(library_config.proxy)
            nc.gpsimd.memset(wmt[:], 1.0)
            nc.gpsimd.memset(v_sb[:, :, :, 64:65], 1.0)
            wm = gm_pool.tile([128, 512], F32, tag="gm", name="wm")
            for _ in range(8):
                nc.tensor.matmul(wm[0:1, 0:448], wmt[0:1, 0:1],
                                 wmt[0:1, :], start=True, stop=True,
                                 skip_group_check=True)
            nc.scalar.activation(warm[:], wmt[0:1, 0:1], AF.Exp,
                                 bias=0.0, scale=1.0)

            # ---- background PE work units -------------------------------
            def qk_chain(t, dst8, w8_sb, ab):
                """fp8 DR projection chain: psum [128, 512] = 64 * (w^T x)
                slice via 4 DoubleRow matmuls (256-contraction blocks),
                then fp8 quantize copy (partition-preserving thanks to the
                host wq/wk column permutation)."""
                o = SQ * t
                ps = gm_pool.tile([128, 512], F32, tag="gm", name="gm")
                for b in range(4):
                    nc.tensor.matmul(
                        ps[:], w8_sb[:, ab, b, :, :],
                        x8_sb[:, b, :, o:o + 512],
                        start=(b == 0), stop=(b == 3), perf_mode=DR)
                nc.vector.tensor_copy(dst8[:, ab, o:o + 512], ps[:])

            def v_chain(t, sc):
                """v projection for kv chunk sc: [128 s, 256 d'] -> v_sb.
                Kept as one unit: its drip slot must precede the diagonal
                pairs of attention(t) that consume it."""
                ps = gm_pool.tile([128, 512], F32, tag="gm", name="gm")
                for ec in range(EC):
                    nc.tensor.matmul(
                        ps[:, 0:256], xT_sb[:, ec, 128 * sc:128 * sc + 128],
                        wv_sb[:, ec, :],
                        start=(ec == 0), stop=(ec == EC - 1))
                nc.vector.tensor_copy(
                    v_sb[:, sc, :, 0:64],
                    ps[:, 0:256].rearrange("p (h d) -> p h d", h=HPC))

            og_slice = {}
            og_done = {}     # slice -> emitted oproj-unit count
            og_flushed = set()

            def oproj_unit(t, et):
                """output projection for s-slice t, e-tile et."""
                o = SQ * t
                ps = gm_pool.tile([128, 512], F32, tag="gm", name="gm")
                for hp in range(2):
                    nc.tensor.matmul(
                        ps[:], wo_sb[:, hp, 128 * et:128 * et + 128],
                        attT[hp][:, o:o + 512],
                        start=(hp == 0), stop=(hp == 1))
                if t == NSL - 1 and et >= 4:
                    # final units: halve the copy latency by splitting
                    # across ACT and DVE
                    nc.scalar.copy(og_slice[t][:, et, 0:256], ps[:, 0:256])
                    nc.vector.tensor_copy(og_slice[t][:, et, 256:512],
                                          ps[:, 256:512])
                elif t == NSL - 1 and et % 2 == 0:
                    nc.scalar.copy(og_slice[t][:, et, :], ps[:])
                else:
                    nc.vector.tensor_copy(og_slice[t][:, et, :], ps[:])
                og_done[t] = og_done.get(t, 0) + 1

            def flush_og():
                # emit output DMAs in half-slices as soon as 4 units are
                # in, so no single 8KB/partition transfer monopolizes the
                # DMA engines near the tail
                for tt in sorted(og_done):
                    done = og_done[tt]
                    for hf in range(2):
                        key = (tt, hf)
                        if done >= 4 * (hf + 1) and key not in og_flushed:
                            og_flushed.add(key)
                            nc.sync.dma_start(
                                oTr[:, 4 * hf:4 * hf + 4,
                                    SQ * tt:SQ * tt + 512],
                                og_slice[tt][:, 4 * hf:4 * hf + 4, :])

            bg = []          # projection chains: drain within their slice
            bgo = []         # oproj units: filler for the ACT-bound slices
            bgo_budget = [0]

            def drip(n):
                for _ in range(n):
                    if bg:
                        bg.pop(0)()
                    elif bgo and bgo_budget[0] > 0:
                        bgo_budget[0] -= 1
                        bgo.pop(0)()

            # ---- attention for query block qb = t, one head-pair -------
            # the two heads of the pair are interleaved chunk-pair by
            # chunk-pair so ACT always has 2 exps queued while PE works
            def attention_hp(t, hp):
                ncols = 4 * (t + 1)          # kv chunks for this query block
                q0 = SQ * t
                pacc = {}
                for hl in range(2):
                    pacc[hl] = pa_pool.tile([65, 512], F32, tag="pa",
                                            name=f"pacc{hl}")
                def emit_attnv(hl, c0, ptile):
                    h = 2 * hp + hl
                    for ci in range(2):
                        c = c0 + ci
                        j = c - 4 * t
                        lo = 128 * j if j > 0 else 0
                        nc.tensor.matmul(
                            pacc[hl][:, lo:512],
                            v_sb[:, c, h, :],
                            ptile[:, ci, lo:512],
                            start=(c == 0), stop=(c == ncols - 1),
                            skip_group_check=True)

                pending = []
                for g in range(ncols // 2):
                    c0 = 2 * g
                    diag2 = (c0 == ncols - 2)     # pair (4t+2, 4t+3)
                    cur = []
                    for hl in range(2):
                        h = 2 * hp + hl
                        hb = 32 * h
                        stp = st_pool.tile([128, 2, 512], F32, tag="st",
                                           name="stp")
                        for ci in range(2):
                            c = c0 + ci
                            j = c - 4 * t         # >=0 only on diagonal band
                            lo = 128 * j if j > 0 else 0
                            nc.tensor.matmul(
                                stp[:, ci, lo:512],
                                k8[hb:hb + 32, :, 128 * c:128 * c + 128],
                                q8[hb:hb + 32, :, q0 + lo:q0 + 512],
                                start=True, stop=True, skip_group_check=True,
                                perf_mode=DR,
                                tile_position=(hb, 0))
                        ptile = pt_pool.tile([128, 2, 512], BF16, tag="pt",
                                             name="ptile")
                        if diag2:
                            # valid sq >= 256 for both chunks of this pair
                            nc.scalar.activation(ptile[:, :, 256:512],
                                                 stp[:, :, 256:512], AF.Exp,
                                                 bias=0.0, scale=EXPSCALE)
                        else:
                            nc.scalar.activation(ptile[:], stp[:], AF.Exp,
                                                 bias=0.0, scale=EXPSCALE)
                        # post-exp causal tri mask on the j-th square (DVE)
                        for ci in range(2):
                            c = c0 + ci
                            j = c - 4 * t
                            if 0 <= j < NB:
                                sl_ = ptile[:, ci, 128 * j:128 * j + 128]
                                nc.vector.tensor_mul(sl_, sl_, tri_sb[:])
                        cur.append((hl, c0, ptile))
                    # lagged attn@V: pairs lag two iterations behind the
                    # scores/exp stream, so these matmuls never wait on ACT
                    pending.append(cur)
                    if len(pending) > 2:
                        for args in pending.pop(0):
                            emit_attnv(*args)
                    drip(2 if t == 0 else 1)
                for cur in pending:
                    for args in cur:
                        emit_attnv(*args)
                # normalize, phase-split so the two heads' chains pipeline:
                # DVE does both recips back-to-back, Pool both broadcasts,
                # then DVE both multiplies.  The final (t, hp) call runs in
                # column halves so the tail oproj can start ~1us earlier.
                halves = ((0, 256), (256, 512)) if (t, hp) == (NSL - 1, 1) \
                    else ((0, 512),)
                for lo, hi in halves:
                    recs, rbs = {}, {}
                    for hl in range(2):
                        recs[hl] = rc_pool.tile([1, 512], F32, tag="rc",
                                                name="rec")
                        nc.vector.reciprocal(recs[hl][:, lo:hi],
                                             pacc[hl][64:65, lo:hi])
                    for hl in range(2):
                        rbs[hl] = rb_pool.tile([64, 512], F32, tag="rb",
                                               name="rb")
                        nc.gpsimd.partition_broadcast(rbs[hl][:, lo:hi],
                                                      recs[hl][0:1, lo:hi],
                                                      channels=64)
                    for hl in range(2):
                        nc.vector.tensor_mul(
                            attT[hp][64 * hl:64 * hl + 64, q0 + lo:q0 + hi],
                            pacc[hl][0:64, lo:hi], rbs[hl][0:64, lo:hi])

            def queue_qk(t):
                for w8_sb, dst8 in ((wq8_sb, q8), (wk8_sb, k8)):
                    for ab in range(2):
                        bg.append(lambda t=t, dst8=dst8, w8_sb=w8_sb, ab=ab:
                                  qk_chain(t, dst8, w8_sb, ab))

            def queue_v(t):
                for sc in range(4 * t, 4 * t + 4):
                    bg.append(lambda t=t, sc=sc: v_chain(t, sc))

            # ---- main slice-pipelined schedule -------------------------
            # qk chains of slice t+1 drain during attention(t); v chains of
            # slice t+1 drain at the START of attention(t+1) (their chunks
            # are only read from pair 2(t+1)); oproj units drip into any
            # window with PE slack (every window is ACT-bound in v3)
            queue_qk(0)
            queue_v(0)
            drip(4)                  # slice 0 q/k chains up front; v chains
            for t in range(NSL):     # drip inside attention(0)
                if t + 1 < NSL:
                    queue_qk(t + 1)
                bgo_budget[0] = (10 * EC if t == NSL - 1 else
                                 8 if t == NSL - 2 else 4)

                og_slice[t] = og_pool.tile([128, EC, 512], BF16, tag="og",
                                           name=f"og{t}")
                for hp in range(2):
                    attention_hp(t, hp)
                    drip(1)
                    flush_og()
                if t + 1 < NSL:
                    queue_v(t + 1)
                # slice t attention done -> queue its output projection
                # as low-priority filler; slice 3's units run right here
                if t == NSL - 1:
                    flush_og()
                    bgo_budget[0] = 10 * EC
                    while bg or bgo:
                        drip(1)
                    flush_og()
                    stx = [st_pool.tile([128, 2, 512], F32, tag="st",
                                        name="stx") for _ in range(2)]
                    # tail oproj: column halves (h2) so the first half's
                    # matmuls start as soon as the half-norm lands; copies
                    # split ACT (low half) / DVE (high half)
                    for et in range(EC):
                        if 2 <= et < 6:
                            ps = stx[(et - 2) // 2][:, et % 2, :]
                        else:
                            ps = gm_pool.tile([128, 512], F32, tag="gm",
                                              name="gm")[:]
                        o = SQ * t
                        for h2 in range(2):
                            c0, c1 = 256 * h2, 256 * h2 + 256
                            for hp_ in range(2):
                                nc.tensor.matmul(
                                    ps[:, c0:c1],
                                    wo_sb[:, hp_, 128 * et:128 * et + 128],
                                    attT[hp_][:, o + c0:o + c1],
                                    start=(hp_ == 0), stop=(hp_ == 1),
                                    skip_group_check=True)
                        nc.scalar.copy(og_slice[t][:, et, 0:256],
                                       ps[:, 0:256])
                        nc.vector.tensor_copy(og_slice[t][:, et, 256:512],
                                              ps[:, 256:512])
                        if et == 5 or et >= 6:
                            lo_et = 4 if et == 5 else et
                            nc.sync.dma_start(
                                oTr[:, lo_et:et + 1, o:o + 512],
                                og_slice[t][:, lo_et:et + 1, :])
                        elif et == 3:
                            nc.sync.dma_start(
                                oTr[:, 0:4, o:o + 512],
                                og_slice[t][:, 0:4, :])
                else:
                    for et in range(EC):
                        bgo.append(lambda t=t, et=et: oproj_unit(t, et))
                    flush_og()

            if debug:
                nc.sync.dma_start(dbg["d_q8"][:, :, :], q8[:])
                nc.sync.dma_start(dbg["d_k8"][:, :, :], k8[:])
                nc.sync.dma_start(
                    dbg["d_v"][:, :],
                    v_sb[:].rearrange("p a b c -> p (a b c)"))
                nc.sync.dma_start(dbg["d_attT0"][:, :], attT[0][:])
                nc.sync.dma_start(dbg["d_attT1"][:, :], attT[1][:])

    nc.compile()
    return nc


def permute_qk_cols(w):
    """[E, 256] -> chain A columns = (h, k) d=k, chain B -> d=32+k."""
    wg = np.asarray(w).reshape(E, HPC, D)
    a = wg[:, :, 0:32].reshape(E, 128)
    b = wg[:, :, 32:64].reshape(E, 128)
    return np.concatenate([a, b], axis=1)


def dr_weight_layout(wperm):
    """[E, 256] permuted -> [128, 2(ab), 4(b), 2(t), 128] fp8 with
    e = 256b + 128t + i."""
    w = np.asarray(wperm, dtype=np.float32) * WSCALE
    w = w.reshape(4, 2, 128, 2, 128)          # (b, t, i, ab, c)
    w = w.transpose(2, 3, 0, 1, 4)            # (i, ab, b, t, c)
    return np.ascontiguousarray(w).astype(NF8)


def dr_x_layout(xT):
    """x[b]^T [E, S] -> [128, 4(b), 2(t), S] fp8 with e = 256b + 128t + i."""
    x = np.asarray(xT, dtype=np.float32).reshape(4, 2, 128, S)
    x = x.transpose(2, 0, 1, 3)
    return np.ascontiguousarray(x).astype(NF8)


_NC_CACHE = None


def kernel(x, w_q, w_k, w_v, w_o):
    global _NC_CACHE
    if _NC_CACHE is None:
        _NC_CACHE = build_kernel()
    nc = _NC_CACHE

    x = np.asarray(x, dtype=np.float32)
    w_q = np.asarray(w_q, dtype=np.float32)
    w_k = np.asarray(w_k, dtype=np.float32)
    w_v = np.asarray(w_v, dtype=np.float32)
    w_o = np.asarray(w_o, dtype=np.float32)

    tri = make_tri()
    in_maps = []
    for core in range(NCORES):
        b, g = divmod(core, NCORES // B)
        sl = slice(g * DP, (g + 1) * DP)
        xT = np.ascontiguousarray(x[b].T)
        in_maps.append({
            "xT": xT.astype(NBF),
            "x8": dr_x_layout(xT),
            "wq8": dr_weight_layout(permute_qk_cols(w_q[:, sl])),
            "wk8": dr_weight_layout(permute_qk_cols(w_k[:, sl])),
            "wv": np.ascontiguousarray(w_v[:, sl]).astype(NBF),
            "wo": np.ascontiguousarray(w_o[sl, :]).astype(NBF),
            "tri": tri,
        })

    res = bass_utils.run_bass_kernel_spmd(nc, in_maps, core_ids=list(range(NCORES)))

    out = np.zeros((B, S, E), dtype=np.float32)
    for core in range(NCORES):
        b = core // (NCORES // B)
        out[b] += res.results[core]["oT"].astype(np.float32).T
    return out


# revision 10
# speedup vs baseline: 1.0317x; 1.0317x over previous
"""Causal self-attention (B=2, S=2048, E=1024, H=16, D=64) on 8 NeuronCores, v3.

Sharding: core = (batch b, head-group g of 4 heads); data parallel on B,
tensor parallel on heads.  Host sums the 4 partial output projections.

v3 vs v2 (126.2us -> target ~88us): q/k projections via full-array fp8
DoubleRow matmuls — host ships x8 [128, 4(b), 2(t), S] fp8 (e = 256b+128t+i)
and wq8/wk8 [128, 2(ab), 4(b), 2(t), 128] fp8 scaled by 64, so each
projection chain is 4 DR matmuls of 256-deep contraction at 0.5 cyc/col
(16.4k PE cyc total vs 65.5k bf16).  Scores psum is scaled 64^2; the exp
scale absorbs it (0.125/4096).  Numerics sim: rel err ~1.5e-2 < 2e-2 gate.

Layouts per core:
  xT_sb [128, 8ec, 2048]  bf16   x[b]^T  (for the v projection, bf16-clean)
  x8_sb [128, 4, 2, 2048] fp8    DR-interleaved x for q/k chains
  wq8/wk8 host-permuted per v2's chain A/B column split (chain ab column
    c = (h in 4, k in 32) -> d = 32*ab + k), fp8 * 64.
  qk chain (t, dst, ab): 4 DR matmuls (256-contraction blocks) -> psum
    [128, 512] = 64*q slice; DVE quantize copy -> q8/k8 [128, 2, S] fp8.
  scores: DR matmul per (head, kv-chunk): stat k8[32h:32h+32, :, 128c:+128],
    mov q8[32h:32h+32, :, 512t:+512] -> stp [128 sk, 512 sq] f32.
  exp on ACT per chunk-pair [128, 1024] f32->bf16, scale=2^-15 fused; no
    max-subtraction (scores provably bounded).  Diagonal-band pairs use a
    half-width exp; causal tri mask multiplied post-exp on DVE.
  attn@V: stat = v_sb[:, c, h, 0:65] (col 64 = ones -> rowsum), mov =
    ptile[:, ci, lo:512] -> pacc [65, 512] f32 accumulated over chunks in
    one PSUM bank; diagonal chunks use a partial moving width (lo = 128j).
  normalize: DVE recip of pacc[64:65, :], gpsimd partition_broadcast to 64
    rows, one DVE multiply writing attT[hp][64*hl:, cols] bf16 directly.
  oproj: per (slice, e-tile): 2 matmuls (hp) wo_sb x attT -> [128, 512] f32,
    copied bf16 and DMA'd out; host converts to f32 and reduces.
"""

import sys

sys.path.insert(0, "/opt/trn_rl_repo")

import numpy as np
import ml_dtypes

import concourse.bass as bass
import concourse.bacc as bacc
import concourse.mybir as mybir
import concourse.tile as tile
from concourse import bass_utils
from concourse import library_config

F32 = mybir.dt.float32
BF16 = mybir.dt.bfloat16
FP8 = mybir.dt.float8e4
AF = mybir.ActivationFunctionType
DR = mybir.MatmulPerfMode.DoubleRow
NBF = ml_dtypes.bfloat16
NF8 = ml_dtypes.float8_e4m3

B, S, E, H, D = 2, 2048, 1024, 16, 64
HPC = 4                 # heads per core
DP = HPC * D            # 256 d' columns per core
NCORES = 8
EC = E // 128           # 8 e-chunks
NSC = S // 128          # 16 kv chunks
NSL = 4                 # s-slices (512 wide); slice t <-> query block qb=t
SQ = 512                # query block width
NB = SQ // 128          # 4 sq sub-blocks per query block
WSCALE = 64.0           # fp8 weight upscale (q/k psum carries 64x)
EXPSCALE = 0.125 / (WSCALE * WSCALE)


def make_tri():
    # multiplicative causal mask for a diagonal 128x128 square of an
    # [sk, sq] tile: keep sq >= sk
    return (np.arange(128)[None, :] >= np.arange(128)[:, None]).astype(NBF)


def build_kernel(debug=False):
    nc = bacc.Bacc("TRN2", target_bir_lowering=False, debug=False)

    xT_d = nc.dram_tensor("xT", [E, S], BF16, kind="ExternalInput")
    x8_d = nc.dram_tensor("x8", [128, 4, 2, S], FP8, kind="ExternalInput")
    wq8_d = nc.dram_tensor("wq8", [128, 2, 4, 2, 128], FP8, kind="ExternalInput")
    wk8_d = nc.dram_tensor("wk8", [128, 2, 4, 2, 128], FP8, kind="ExternalInput")
    wv_d = nc.dram_tensor("wv", [E, DP], BF16, kind="ExternalInput")
    wo_d = nc.dram_tensor("wo", [DP, E], BF16, kind="ExternalInput")
    tri_d = nc.dram_tensor("tri", [128, 128], BF16, kind="ExternalInput")
    oT_d = nc.dram_tensor("oT", [E, S], BF16, kind="ExternalOutput")
    if debug:
        dbg = {
            "d_q8": nc.dram_tensor("d_q8", [128, 2, S], FP8, kind="ExternalOutput"),
            "d_k8": nc.dram_tensor("d_k8", [128, 2, S], FP8, kind="ExternalOutput"),
            "d_v": nc.dram_tensor("d_v", [128, NSC * HPC * 65], BF16, kind="ExternalOutput"),
            "d_attT0": nc.dram_tensor("d_attT0", [128, S], BF16, kind="ExternalOutput"),
            "d_attT1": nc.dram_tensor("d_attT1", [128, S], BF16, kind="ExternalOutput"),
        }

    xTr = xT_d.rearrange("(c p) s -> p c s", p=128)
    wvr = wv_d.rearrange("(c p) d -> p c d", p=128)
    wor = wo_d.rearrange("(c p) e -> p c e", p=128)
    oTr = oT_d.rearrange("(a p) s -> p a s", p=128)

    with tile.TileContext(nc) as tc:
        with tc.tile_pool(name="big", bufs=1) as big, \
             tc.tile_pool(name="pt", bufs=10) as pt_pool, \
             tc.tile_pool(name="rc", bufs=4) as rc_pool, \
             tc.tile_pool(name="rb", bufs=4) as rb_pool, \
             tc.tile_pool(name="og", bufs=4) as og_pool, \
             tc.tile_pool(name="st", bufs=2, space="PSUM") as st_pool, \
             tc.tile_pool(name="pa", bufs=2, space="PSUM") as pa_pool, \
             tc.tile_pool(name="gm", bufs=2, space="PSUM") as gm_pool:

            xT_sb = big.tile([128, EC, S], BF16, tag="xT", name="xT_sb")
            x8_sb = big.tile([128, 4, 2, S], FP8, tag="x8", name="x8_sb")
            wq8_sb = big.tile([128, 2, 4, 2, 128], FP8, tag="wq8", name="wq8_sb")
            wk8_sb = big.tile([128, 2, 4, 2, 128], FP8, tag="wk8", name="wk8_sb")
            wv_sb = big.tile([128, EC, DP], BF16, tag="wv", name="wv_sb")
            wo_sb = big.tile([128, 2, E], BF16, tag="wo", name="wo_sb")
            q8 = big.tile([128, 2, S], FP8, tag="q8", name="q8")
            k8 = big.tile([128, 2, S], FP8, tag="k8", name="k8")
            v_sb = big.tile([128, NSC, HPC, 65], BF16, tag="v", name="v_sb")
            attT = [big.tile([128, S], BF16, tag=f"attT{i}", name=f"attT{i}")
                    for i in range(2)]
            tri_sb = big.tile([128, 128], BF16, tag="tri", name="tri_sb")
            wmt = big.tile([1, 448], BF16, tag="wmt", name="wmt")
            warm = big.tile([1, 1], F32, tag="warm", name="warm")

            # ---- preamble: DMAs first (sync ring is serial — order by
            # first-consumer time), then gpsimd library + exp/PE warm ----
            nc.sync.dma_start(x8_sb[:, :, :, 0:512], x8_d[:, :, :, 0:512])
            nc.sync.dma_start(wq8_sb[:], wq8_d[:, :, :, :, :])
            nc.sync.dma_start(wk8_sb[:], wk8_d[:, :, :, :, :])
            nc.sync.dma_start(tri_sb[:], tri_d[:, :])
            nc.sync.dma_start(xT_sb[:, :, 0:256], xTr[:, :, 0:256])
            nc.sync.dma_start(wv_sb[:], wvr[:, :, :])
            nc.sync.dma_start(xT_sb[:, :, 256:512], xTr[:, :, 256:512])
            nc.sync.dma_start(x8_sb[:, :, :, 512:1024], x8_d[:, :, :, 512:1024])
            nc.sync.dma_start(xT_sb[:, :, 512:1024], xTr[:, :, 512:1024])
            nc.sync.dma_start(x8_sb[:, :, :, 1024:2048], x8_d[:, :, :, 1024:2048])
            nc.sync.dma_start(wo_sb[:], wor[:, :, :])
            nc.sync.dma_start(xT_sb[:, :, 1024:1536], xTr[:, :, 1024:1536])
            nc.sync.dma_start(xT_sb[:, :, 1536:2048], xTr[:, :, 1536:2048])
            nc.gpsimd.load_library(library_config.proxy)
            nc.gpsimd.memset(wmt[:], 1.0)
            nc.gpsimd.memset(v_sb[:, :, :, 64:65], 1.0)
            wm = gm_pool.tile([128, 512], F32, tag="gm", name="wm")
            for _ in range(8):
                nc.tensor.matmul(wm[0:1, 0:448], wmt[0:1, 0:1],
                                 wmt[0:1, :], start=True, stop=True,
                                 skip_group_check=True)
            nc.scalar.activation(warm[:], wmt[0:1, 0:1], AF.Exp,
                                 bias=0.0, scale=1.0)

            # ---- background PE work units -------------------------------
            def qk_chain(t, dst8, w8_sb, ab):
                """fp8 DR projection chain: psum [128, 512] = 64 * (w^T x)
                slice via 4 DoubleRow matmuls (256-contraction blocks),
                then fp8 quantize copy (partition-preserving thanks to the
                host wq/wk column permutation)."""
                o = SQ * t
                ps = gm_pool.tile([128, 512], F32, tag="gm", name="gm")
                for b in range(4):
                    nc.tensor.matmul(
                        ps[:], w8_sb[:, ab, b, :, :],
                        x8_sb[:, b, :, o:o + 512],
                        start=(b == 0), stop=(b == 3), perf_mode=DR)
                nc.vector.tensor_copy(dst8[:, ab, o:o + 512], ps[:])

            def v_chain(t, sc):
                """v projection for kv chunk sc: [128 s, 256 d'] -> v_sb.
                Kept as one unit: its drip slot must precede the diagonal
                pairs of attention(t) that consume it."""
                ps = gm_pool.tile([128, 512], F32, tag="gm", name="gm")
                for ec in range(EC):
                    nc.tensor.matmul(
                        ps[:, 0:256], xT_sb[:, ec, 128 * sc:128 * sc + 128],
                        wv_sb[:, ec, :],
                        start=(ec == 0), stop=(ec == EC - 1))
                nc.vector.tensor_copy(
                    v_sb[:, sc, :, 0:64],
                    ps[:, 0:256].rearrange("p (h d) -> p h d", h=HPC))

            og_slice = {}
            og_done = {}     # slice -> emitted oproj-unit count
            og_flushed = set()

            def oproj_unit(t, et):
                """output projection for s-slice t, e-tile et."""
                o = SQ * t
                ps = gm_pool.tile([128, 512], F32, tag="gm", name="gm")
                for hp in range(2):
                    nc.tensor.matmul(
                        ps[:], wo_sb[:, hp, 128 * et:128 * et + 128],
                        attT[hp][:, o:o + 512],
                        start=(hp == 0), stop=(hp == 1))
                if t == NSL - 1 and et >= 4:
                    # final units: halve the copy latency by splitting
                    # across ACT and DVE
                    nc.scalar.copy(og_slice[t][:, et, 0:256], ps[:, 0:256])
                    nc.vector.tensor_copy(og_slice[t][:, et, 256:512],
                                          ps[:, 256:512])
                elif t == NSL - 1 and et % 2 == 0:
                    nc.scalar.copy(og_slice[t][:, et, :], ps[:])
                else:
                    nc.vector.tensor_copy(og_slice[t][:, et, :], ps[:])
                og_done[t] = og_done.get(t, 0) + 1

            def flush_og():
                # emit output DMAs in half-slices as soon as 4 units are
                # in, so no single 8KB/partition transfer monopolizes the
                # DMA engines near the tail
                for tt in sorted(og_done):
                    done = og_done[tt]
                    for hf in range(2):
                        key = (tt, hf)
                        if done >= 4 * (hf + 1) and key not in og_flushed:
                            og_flushed.add(key)
                            nc.sync.dma_start(
                                oTr[:, 4 * hf:4 * hf + 4,
                                    SQ * tt:SQ * tt + 512],
                                og_slice[tt][:, 4 * hf:4 * hf + 4, :])

            bg = []          # projection chains: drain within their slice
            bgo = []         # oproj units: filler for the ACT-bound slices
            bgo_budget = [0]

            def drip(n):
                for _ in range(n):
                    if bg:
                        bg.pop(0)()
                    elif bgo and bgo_budget[0] > 0:
                        bgo_budget[0] -= 1
                        bgo.pop(0)()

            # ---- attention for query block qb = t, one head-pair -------
            # the two heads of the pair are interleaved chunk-pair by
            # chunk-pair so ACT always has 2 exps queued while PE works.
            # Split into head (scores/exp g-loop) and tail (attn@V flush +
            # normalize): the PREVIOUS pair's tail is injected after this
            # pair's second g so the exp stream never stalls at boundaries.
            def attn_head(t, hp, inject):
                ncols = 4 * (t + 1)          # kv chunks for this query block
                q0 = SQ * t
                pacc = {}
                for hl in range(2):
                    pacc[hl] = pa_pool.tile([65, 512], F32, tag="pa",
                                            name=f"pacc{hl}")
                def emit_attnv(hl, c0, ptile):
                    h = 2 * hp + hl
                    for ci in range(2):
                        c = c0 + ci
                        j = c - 4 * t
                        lo = 128 * j if j > 0 else 0
                        nc.tensor.matmul(
                            pacc[hl][:, lo:512],
                            v_sb[:, c, h, :],
                            ptile[:, ci, lo:512],
                            start=(c == 0), stop=(c == ncols - 1),
                            skip_group_check=True)

                pending = []
                for g in range(ncols // 2):
                    c0 = 2 * g
                    diag2 = (c0 == ncols - 2)     # pair (4t+2, 4t+3)
                    cur = []
                    for hl in range(2):
                        h = 2 * hp + hl
                        hb = 32 * h
                        stp = st_pool.tile([128, 2, 512], F32, tag="st",
                                           name="stp")
                        for ci in range(2):
                            c = c0 + ci
                            j = c - 4 * t         # >=0 only on diagonal band
                            lo = 128 * j if j > 0 else 0
                            nc.tensor.matmul(
                                stp[:, ci, lo:512],
                                k8[hb:hb + 32, :, 128 * c:128 * c + 128],
                                q8[hb:hb + 32, :, q0 + lo:q0 + 512],
                                start=True, stop=True, skip_group_check=True,
                                perf_mode=DR,
                                tile_position=(hb, 0))
                        ptile = pt_pool.tile([128, 2, 512], BF16, tag="pt",
                                             name="ptile")
                        if diag2:
                            # valid sq >= 256 for both chunks of this pair
                            nc.scalar.activation(ptile[:, :, 256:512],
                                                 stp[:, :, 256:512], AF.Exp,
                                                 bias=0.0, scale=EXPSCALE)
                        else:
                            nc.scalar.activation(ptile[:], stp[:], AF.Exp,
                                                 bias=0.0, scale=EXPSCALE)
                        # post-exp causal tri mask on the j-th square (DVE)
                        for ci in range(2):
                            c = c0 + ci
                            j = c - 4 * t
                            if 0 <= j < NB:
                                sl_ = ptile[:, ci, 128 * j:128 * j + 128]
                                nc.vector.tensor_mul(sl_, sl_, tri_sb[:])
                        cur.append((hl, c0, ptile))
                    # lagged attn@V: pairs lag two iterations behind the
                    # scores/exp stream, so these matmuls never wait on ACT
                    pending.append(cur)
                    if len(pending) > 2:
                        for args in pending.pop(0):
                            emit_attnv(*args)
                    drip(2 if t == 0 else 1)
                    if g == min(1, ncols // 2 - 1) and inject is not None:
                        inject()
                        inject = None
                if inject is not None:
                    inject()
                return lambda: attn_tail(t, hp, pacc, pending, emit_attnv)

            def attn_tail(t, hp, pacc, pending, emit_attnv):
                q0 = SQ * t
                for cur in pending:
                    for args in cur:
                        emit_attnv(*args)
                # normalize, phase-split so the two heads' chains pipeline:
                # DVE does both recips back-to-back, Pool both broadcasts,
                # then DVE both multiplies.  The final (t, hp) call runs in
                # column halves so the tail oproj can start ~1us earlier.
                halves = ((0, 256), (256, 512)) if (t, hp) == (NSL - 1, 1) \
                    else ((0, 512),)
                for lo, hi in halves:
                    recs, rbs = {}, {}
                    for hl in range(2):
                        recs[hl] = rc_pool.tile([1, 512], F32, tag="rc",
                                                name="rec")
                        nc.vector.reciprocal(recs[hl][:, lo:hi],
                                             pacc[hl][64:65, lo:hi])
                    for hl in range(2):
                        rbs[hl] = rb_pool.tile([64, 512], F32, tag="rb",
                                               name="rb")
                        nc.gpsimd.partition_broadcast(rbs[hl][:, lo:hi],
                                                      recs[hl][0:1, lo:hi],
                                                      channels=64)
                    for hl in range(2):
                        nc.vector.tensor_mul(
                            attT[hp][64 * hl:64 * hl + 64, q0 + lo:q0 + hi],
                            pacc[hl][0:64, lo:hi], rbs[hl][0:64, lo:hi])

            def queue_qk(t):
                for w8_sb, dst8 in ((wq8_sb, q8), (wk8_sb, k8)):
                    for ab in range(2):
                        bg.append(lambda t=t, dst8=dst8, w8_sb=w8_sb, ab=ab:
                                  qk_chain(t, dst8, w8_sb, ab))

            def queue_v(t):
                for sc in range(4 * t, 4 * t + 4):
                    bg.append(lambda t=t, sc=sc: v_chain(t, sc))

            # ---- main slice-pipelined schedule -------------------------
            # qk chains of slice t+1 drain during attention(t); v chains of
            # slice t+1 drain at the START of attention(t+1) (their chunks
            # are only read from pair 2(t+1)); oproj units drip into any
            # window with PE slack (every window is ACT-bound in v3)
            queue_qk(0)
            queue_v(0)
            drip(4)                  # slice 0 q/k chains up front; v chains
            tail = None              # drip inside attention(0)
            for t in range(NSL):
                if t + 1 < NSL:
                    queue_qk(t + 1)
                bgo_budget[0] = (10 * EC if t == NSL - 1 else
                                 10 if t == NSL - 2 else 8)

                og_slice[t] = og_pool.tile([128, EC, 512], BF16, tag="og",
                                           name=f"og{t}")
                for hp in range(2):
                    tail = attn_head(t, hp, tail)
                    drip(1)
                    flush_og()
                if t + 1 < NSL:
                    queue_v(t + 1)
                # slice t attention done -> queue its output projection
                # as low-priority filler; slice 3's units run right here
                if t == NSL - 1:
                    tail()           # final pair's attn@V flush + norm
                    tail = None
                    flush_og()
                    bgo_budget[0] = 10 * EC
                    while bg or bgo:
                        drip(1)
                    flush_og()
                    stx = [st_pool.tile([128, 2, 512], F32, tag="st",
                                        name="stx") for _ in range(2)]
                    # tail oproj: column halves (h2) so the first half's
                    # matmuls start as soon as the half-norm lands; copies
                    # split ACT (low half) / DVE (high half)
                    for et in range(EC):
                        if 2 <= et < 6:
                            ps = stx[(et - 2) // 2][:, et % 2, :]
                        else:
                            ps = gm_pool.tile([128, 512], F32, tag="gm",
                                              name="gm")[:]
                        o = SQ * t
                        for h2 in range(2):
                            c0, c1 = 256 * h2, 256 * h2 + 256
                            for hp_ in range(2):
                                nc.tensor.matmul(
                                    ps[:, c0:c1],
                                    wo_sb[:, hp_, 128 * et:128 * et + 128],
                                    attT[hp_][:, o + c0:o + c1],
                                    start=(hp_ == 0), stop=(hp_ == 1),
                                    skip_group_check=True)
                        nc.scalar.copy(og_slice[t][:, et, 0:256],
                                       ps[:, 0:256])
                        nc.vector.tensor_copy(og_slice[t][:, et, 256:512],
                                              ps[:, 256:512])
                        if et == 5 or et >= 6:
                            lo_et = 4 if et == 5 else et
                            nc.sync.dma_start(
                                oTr[:, lo_et:et + 1, o:o + 512],
                                og_slice[t][:, lo_et:et + 1, :])
                        elif et == 3:
                            nc.sync.dma_start(
                                oTr[:, 0:4, o:o + 512],
                                og_slice[t][:, 0:4, :])
                else:
                    for et in range(EC):
                        bgo.append(lambda t=t, et=et: oproj_unit(t, et))
                    flush_og()

            if debug:
                nc.sync.dma_start(dbg["d_q8"][:, :, :], q8[:])
                nc.sync.dma_start(dbg["d_k8"][:, :, :], k8[:])
                nc.sync.dma_start(
                    dbg["d_v"][:, :],
                    v_sb[:].rearrange("p a b c -> p (a b c)"))
                nc.sync.dma_start(dbg["d_attT0"][:, :], attT[0][:])
                nc.sync.dma_start(dbg["d_attT1"][:, :], attT[1][:])

    nc.compile()
    return nc


def permute_qk_cols(w):
    """[E, 256] -> chain A columns = (h, k) d=k, chain B -> d=32+k."""
    wg = np.asarray(w).reshape(E, HPC, D)
    a = wg[:, :, 0:32].reshape(E, 128)
    b = wg[:, :, 32:64].reshape(E, 128)
    return np.concatenate([a, b], axis=1)


def dr_weight_layout(wperm):
    """[E, 256] permuted -> [128, 2(ab), 4(b), 2(t), 128] fp8 with
    e = 256b + 128t + i."""
    w = np.asarray(wperm, dtype=np.float32) * WSCALE
    w = w.reshape(4, 2, 128, 2, 128)          # (b, t, i, ab, c)
    w = w.transpose(2, 3, 0, 1, 4)            # (i, ab, b, t, c)
    return np.ascontiguousarray(w).astype(NF8)


def dr_x_layout(xT):
    """x[b]^T [E, S] -> [128, 4(b), 2(t), S] fp8 with e = 256b + 128t + i."""
    x = np.asarray(xT, dtype=np.float32).reshape(4, 2, 128, S)
    x = x.transpose(2, 0, 1, 3)
    return np.ascontiguousarray(x).astype(NF8)


_NC_CACHE = None


def kernel(x, w_q, w_k, w_v, w_o):
    global _NC_CACHE
    if _NC_CACHE is None:
        _NC_CACHE = build_kernel()
    nc = _NC_CACHE

    x = np.asarray(x, dtype=np.float32)
    w_q = np.asarray(w_q, dtype=np.float32)
    w_k = np.asarray(w_k, dtype=np.float32)
    w_v = np.asarray(w_v, dtype=np.float32)
    w_o = np.asarray(w_o, dtype=np.float32)

    tri = make_tri()
    in_maps = []
    for core in range(NCORES):
        b, g = divmod(core, NCORES // B)
        sl = slice(g * DP, (g + 1) * DP)
        xT = np.ascontiguousarray(x[b].T)
        in_maps.append({
            "xT": xT.astype(NBF),
            "x8": dr_x_layout(xT),
            "wq8": dr_weight_layout(permute_qk_cols(w_q[:, sl])),
            "wk8": dr_weight_layout(permute_qk_cols(w_k[:, sl])),
            "wv": np.ascontiguousarray(w_v[:, sl]).astype(NBF),
            "wo": np.ascontiguousarray(w_o[sl, :]).astype(NBF),
            "tri": tri,
        })

    res = bass_utils.run_bass_kernel_spmd(nc, in_maps, core_ids=list(range(NCORES)))

    out = np.zeros((B, S, E), dtype=np.float32)
    for core in range(NCORES):
        b = core // (NCORES // B)
        out[b] += res.results[core]["oT"].astype(np.float32).T
    return out


# revision 14
# speedup vs baseline: 1.0405x; 1.0085x over previous
"""Causal self-attention (B=2, S=2048, E=1024, H=16, D=64) on 8 NeuronCores, v3.

Sharding: core = (batch b, head-group g of 4 heads); data parallel on B,
tensor parallel on heads.  Host sums the 4 partial output projections.

v3 vs v2 (126.2us -> target ~88us): q/k projections via full-array fp8
DoubleRow matmuls — host ships x8 [128, 4(b), 2(t), S] fp8 (e = 256b+128t+i)
and wq8/wk8 [128, 2(ab), 4(b), 2(t), 128] fp8 scaled by 64, so each
projection chain is 4 DR matmuls of 256-deep contraction at 0.5 cyc/col
(16.4k PE cyc total vs 65.5k bf16).  Scores psum is scaled 64^2; the exp
scale absorbs it (0.125/4096).  Numerics sim: rel err ~1.5e-2 < 2e-2 gate.

Layouts per core:
  xT_sb [128, 8ec, 2048]  bf16   x[b]^T  (for the v projection, bf16-clean)
  x8_sb [128, 4, 2, 2048] fp8    DR-interleaved x for q/k chains
  wq8/wk8 host-permuted per v2's chain A/B column split (chain ab column
    c = (h in 4, k in 32) -> d = 32*ab + k), fp8 * 64.
  qk chain (t, dst, ab): 4 DR matmuls (256-contraction blocks) -> psum
    [128, 512] = 64*q slice; DVE quantize copy -> q8/k8 [128, 2, S] fp8.
  scores: DR matmul per (head, kv-chunk): stat k8[32h:32h+32, :, 128c:+128],
    mov q8[32h:32h+32, :, 512t:+512] -> stp [128 sk, 512 sq] f32.
  exp on ACT per chunk-pair [128, 1024] f32->bf16, scale=2^-15 fused; no
    max-subtraction (scores provably bounded).  Diagonal-band pairs use a
    half-width exp; causal tri mask multiplied post-exp on DVE.
  attn@V: stat = v_sb[:, c, h, 0:65] (col 64 = ones -> rowsum), mov =
    ptile[:, ci, lo:512] -> pacc [65, 512] f32 accumulated over chunks in
    one PSUM bank; diagonal chunks use a partial moving width (lo = 128j).
  normalize: DVE recip of pacc[64:65, :], gpsimd partition_broadcast to 64
    rows, one DVE multiply writing attT[hp][64*hl:, cols] bf16 directly.
  oproj: per (slice, e-tile): 2 matmuls (hp) wo_sb x attT -> [128, 512] f32,
    copied bf16 and DMA'd out; host converts to f32 and reduces.
"""

import sys

sys.path.insert(0, "/opt/trn_rl_repo")

import numpy as np
import ml_dtypes

import concourse.bass as bass
import concourse.bacc as bacc
import concourse.mybir as mybir
import concourse.tile as tile
from concourse import bass_utils
from concourse import library_config

F32 = mybir.dt.float32
BF16 = mybir.dt.bfloat16
FP8 = mybir.dt.float8e4
AF = mybir.ActivationFunctionType
DR = mybir.MatmulPerfMode.DoubleRow
NBF = ml_dtypes.bfloat16
NF8 = ml_dtypes.float8_e4m3

B, S, E, H, D = 2, 2048, 1024, 16, 64
HPC = 4                 # heads per core
DP = HPC * D            # 256 d' columns per core
NCORES = 8
EC = E // 128           # 8 e-chunks
NSC = S // 128          # 16 kv chunks
NSL = 4                 # s-slices (512 wide); slice t <-> query block qb=t
SQ = 512                # query block width
NB = SQ // 128          # 4 sq sub-blocks per query block
WSCALE = 64.0           # fp8 weight upscale (q/k psum carries 64x)
EXPSCALE = 0.125 / (WSCALE * WSCALE)


def make_tri():
    # multiplicative causal mask for a diagonal 128x128 square of an
    # [sk, sq] tile: keep sq >= sk
    return (np.arange(128)[None, :] >= np.arange(128)[:, None]).astype(NBF)


def build_kernel(debug=False):
    nc = bacc.Bacc("TRN2", target_bir_lowering=False, debug=False)

    xT_d = nc.dram_tensor("xT", [E, S], BF16, kind="ExternalInput")
    x8_d = nc.dram_tensor("x8", [128, 4, 2, S], FP8, kind="ExternalInput")
    wq8_d = nc.dram_tensor("wq8", [128, 2, 4, 2, 128], FP8, kind="ExternalInput")
    wk8_d = nc.dram_tensor("wk8", [128, 2, 4, 2, 128], FP8, kind="ExternalInput")
    wv_d = nc.dram_tensor("wv", [E, DP], BF16, kind="ExternalInput")
    wo_d = nc.dram_tensor("wo", [DP, E], BF16, kind="ExternalInput")
    tri_d = nc.dram_tensor("tri", [128, 128], BF16, kind="ExternalInput")
    oT_d = nc.dram_tensor("oT", [E, S], BF16, kind="ExternalOutput")
    if debug:
        dbg = {
            "d_q8": nc.dram_tensor("d_q8", [128, 2, S], FP8, kind="ExternalOutput"),
            "d_k8": nc.dram_tensor("d_k8", [128, 2, S], FP8, kind="ExternalOutput"),
            "d_v": nc.dram_tensor("d_v", [128, NSC * HPC * 65], BF16, kind="ExternalOutput"),
            "d_attT0": nc.dram_tensor("d_attT0", [128, S], BF16, kind="ExternalOutput"),
            "d_attT1": nc.dram_tensor("d_attT1", [128, S], BF16, kind="ExternalOutput"),
        }

    xTr = xT_d.rearrange("(c p) s -> p c s", p=128)
    wvr = wv_d.rearrange("(c p) d -> p c d", p=128)
    wor = wo_d.rearrange("(c p) e -> p c e", p=128)
    oTr = oT_d.rearrange("(a p) s -> p a s", p=128)

    with tile.TileContext(nc) as tc:
        with tc.tile_pool(name="big", bufs=1) as big, \
             tc.tile_pool(name="pt", bufs=10) as pt_pool, \
             tc.tile_pool(name="rc", bufs=4) as rc_pool, \
             tc.tile_pool(name="rb", bufs=4) as rb_pool, \
             tc.tile_pool(name="og", bufs=4) as og_pool, \
             tc.tile_pool(name="st", bufs=2, space="PSUM") as st_pool, \
             tc.tile_pool(name="pa", bufs=2, space="PSUM") as pa_pool, \
             tc.tile_pool(name="gm", bufs=2, space="PSUM") as gm_pool:

            xT_sb = big.tile([128, EC, S], BF16, tag="xT", name="xT_sb")
            x8_sb = big.tile([128, 4, 2, S], FP8, tag="x8", name="x8_sb")
            wq8_sb = big.tile([128, 2, 4, 2, 128], FP8, tag="wq8", name="wq8_sb")
            wk8_sb = big.tile([128, 2, 4, 2, 128], FP8, tag="wk8", name="wk8_sb")
            wv_sb = big.tile([128, EC, DP], BF16, tag="wv", name="wv_sb")
            wo_sb = big.tile([128, 2, E], BF16, tag="wo", name="wo_sb")
            q8 = big.tile([128, 2, S], FP8, tag="q8", name="q8")
            k8 = big.tile([128, 2, S], FP8, tag="k8", name="k8")
            v_sb = big.tile([128, NSC, HPC, 65], BF16, tag="v", name="v_sb")
            attT = [big.tile([128, S], BF16, tag=f"attT{i}", name=f"attT{i}")
                    for i in range(2)]
            tri_sb = big.tile([128, 128], BF16, tag="tri", name="tri_sb")
            wmt = big.tile([1, 448], BF16, tag="wmt", name="wmt")
            warm = big.tile([1, 1], F32, tag="warm", name="warm")

            # ---- preamble: DMAs first (sync ring is serial — order by
            # first-consumer time), then gpsimd library + exp/PE warm ----
            nc.sync.dma_start(x8_sb[:, :, :, 0:512], x8_d[:, :, :, 0:512])
            nc.sync.dma_start(wq8_sb[:], wq8_d[:, :, :, :, :])
            nc.sync.dma_start(wk8_sb[:], wk8_d[:, :, :, :, :])
            nc.sync.dma_start(tri_sb[:], tri_d[:, :])
            nc.sync.dma_start(xT_sb[:, :, 0:256], xTr[:, :, 0:256])
            nc.sync.dma_start(wv_sb[:], wvr[:, :, :])
            nc.sync.dma_start(xT_sb[:, :, 256:512], xTr[:, :, 256:512])
            nc.sync.dma_start(x8_sb[:, :, :, 512:1024], x8_d[:, :, :, 512:1024])
            nc.sync.dma_start(xT_sb[:, :, 512:1024], xTr[:, :, 512:1024])
            nc.sync.dma_start(x8_sb[:, :, :, 1024:2048], x8_d[:, :, :, 1024:2048])
            nc.sync.dma_start(wo_sb[:], wor[:, :, :])
            nc.sync.dma_start(xT_sb[:, :, 1024:1536], xTr[:, :, 1024:1536])
            nc.sync.dma_start(xT_sb[:, :, 1536:2048], xTr[:, :, 1536:2048])
            nc.gpsimd.load_library(library_config.proxy)
            nc.gpsimd.memset(wmt[:], 1.0)
            nc.gpsimd.memset(v_sb[:, :, :, 64:65], 1.0)
            wm = gm_pool.tile([128, 512], F32, tag="gm", name="wm")
            for _ in range(8):
                nc.tensor.matmul(wm[0:1, 0:448], wmt[0:1, 0:1],
                                 wmt[0:1, :], start=True, stop=True,
                                 skip_group_check=True)
            nc.scalar.activation(warm[:], wmt[0:1, 0:1], AF.Exp,
                                 bias=0.0, scale=1.0)

            # ---- background PE work units -------------------------------
            def qk_chain(t, dst8, w8_sb, ab):
                """fp8 DR projection chain: psum [128, 512] = 64 * (w^T x)
                slice via 4 DoubleRow matmuls (256-contraction blocks),
                then fp8 quantize copy (partition-preserving thanks to the
                host wq/wk column permutation)."""
                o = SQ * t
                ps = gm_pool.tile([128, 512], F32, tag="gm", name="gm")
                for b in range(4):
                    nc.tensor.matmul(
                        ps[:], w8_sb[:, ab, b, :, :],
                        x8_sb[:, b, :, o:o + 512],
                        start=(b == 0), stop=(b == 3), perf_mode=DR)
                if t == 0 and dst8 is q8:
                    # startup: ACT is idle, halve the 4-copy DVE serial
                    # chain ahead of the very first scores
                    nc.scalar.copy(dst8[:, ab, o:o + 512], ps[:])
                else:
                    nc.vector.tensor_copy(dst8[:, ab, o:o + 512], ps[:])

            def v_chain(t, sc):
                """v projection for kv chunk sc: [128 s, 256 d'] -> v_sb.
                Kept as one unit: its drip slot must precede the diagonal
                pairs of attention(t) that consume it."""
                ps = gm_pool.tile([128, 512], F32, tag="gm", name="gm")
                for ec in range(EC):
                    nc.tensor.matmul(
                        ps[:, 0:256], xT_sb[:, ec, 128 * sc:128 * sc + 128],
                        wv_sb[:, ec, :],
                        start=(ec == 0), stop=(ec == EC - 1))
                nc.vector.tensor_copy(
                    v_sb[:, sc, :, 0:64],
                    ps[:, 0:256].rearrange("p (h d) -> p h d", h=HPC))

            og_slice = {}
            og_done = {}     # slice -> emitted oproj-unit count
            og_flushed = set()

            def oproj_unit(t, et):
                """output projection for s-slice t, e-tile et."""
                o = SQ * t
                ps = gm_pool.tile([128, 512], F32, tag="gm", name="gm")
                for hp in range(2):
                    nc.tensor.matmul(
                        ps[:], wo_sb[:, hp, 128 * et:128 * et + 128],
                        attT[hp][:, o:o + 512],
                        start=(hp == 0), stop=(hp == 1))
                if t == NSL - 1 and et >= 4:
                    # final units: halve the copy latency by splitting
                    # across ACT and DVE
                    nc.scalar.copy(og_slice[t][:, et, 0:256], ps[:, 0:256])
                    nc.vector.tensor_copy(og_slice[t][:, et, 256:512],
                                          ps[:, 256:512])
                elif t == NSL - 1 and et % 2 == 0:
                    nc.scalar.copy(og_slice[t][:, et, :], ps[:])
                else:
                    nc.vector.tensor_copy(og_slice[t][:, et, :], ps[:])
                og_done[t] = og_done.get(t, 0) + 1

            def flush_og():
                # emit output DMAs in half-slices as soon as 4 units are
                # in, so no single 8KB/partition transfer monopolizes the
                # DMA engines near the tail
                for tt in sorted(og_done):
                    done = og_done[tt]
                    for hf in range(2):
                        key = (tt, hf)
                        if done >= 4 * (hf + 1) and key not in og_flushed:
                            og_flushed.add(key)
                            nc.sync.dma_start(
                                oTr[:, 4 * hf:4 * hf + 4,
                                    SQ * tt:SQ * tt + 512],
                                og_slice[tt][:, 4 * hf:4 * hf + 4, :])

            bg = []          # projection chains: drain within their slice
            bgo = []         # oproj units: filler for the ACT-bound slices
            bgo_budget = [0]

            def drip(n):
                for _ in range(n):
                    if bg:
                        bg.pop(0)()
                    elif bgo and bgo_budget[0] > 0:
                        bgo_budget[0] -= 1
                        bgo.pop(0)()

            # ---- attention for query block qb = t, one head-pair -------
            # the two heads of the pair are interleaved chunk-pair by
            # chunk-pair so ACT always has 2 exps queued while PE works.
            # Split into head (scores/exp g-loop) and tail (attn@V flush +
            # normalize): the PREVIOUS pair's tail is injected after this
            # pair's second g so the exp stream never stalls at boundaries.
            def attn_head(t, hp, inject):
                ncols = 4 * (t + 1)          # kv chunks for this query block
                q0 = SQ * t
                pacc = {}
                for hl in range(2):
                    pacc[hl] = pa_pool.tile([65, 512], F32, tag="pa",
                                            name=f"pacc{hl}")
                def emit_attnv_chunk(hl, c0, ptile, ci):
                    h = 2 * hp + hl
                    c = c0 + ci
                    j = c - 4 * t
                    lo = 128 * j if j > 0 else 0
                    nc.tensor.matmul(
                        pacc[hl][:, lo:512],
                        v_sb[:, c, h, :],
                        ptile[:, ci, lo:512],
                        start=(c == 0), stop=(c == ncols - 1),
                        skip_group_check=True)

                def emit_attnv(hl, c0, ptile):
                    for ci in range(2):
                        emit_attnv_chunk(hl, c0, ptile, ci)

                pending = []
                for g in range(ncols // 2):
                    c0 = 2 * g
                    diag2 = (c0 == ncols - 2)     # pair (4t+2, 4t+3)
                    cur = []
                    for hl in range(2):
                        h = 2 * hp + hl
                        hb = 32 * h
                        stp = st_pool.tile([128, 2, 512], F32, tag="st",
                                           name="stp")
                        for ci in range(2):
                            c = c0 + ci
                            j = c - 4 * t         # >=0 only on diagonal band
                            lo = 128 * j if j > 0 else 0
                            nc.tensor.matmul(
                                stp[:, ci, lo:512],
                                k8[hb:hb + 32, :, 128 * c:128 * c + 128],
                                q8[hb:hb + 32, :, q0 + lo:q0 + 512],
                                start=True, stop=True, skip_group_check=True,
                                perf_mode=DR,
                                tile_position=(hb, 0))
                        ptile = pt_pool.tile([128, 2, 512], BF16, tag="pt",
                                             name="ptile")
                        if diag2:
                            # valid sq >= 256 for both chunks of this pair
                            nc.scalar.activation(ptile[:, :, 256:512],
                                                 stp[:, :, 256:512], AF.Exp,
                                                 bias=0.0, scale=EXPSCALE)
                        else:
                            nc.scalar.activation(ptile[:], stp[:], AF.Exp,
                                                 bias=0.0, scale=EXPSCALE)
                        # post-exp causal tri mask on the j-th square (DVE)
                        for ci in range(2):
                            c = c0 + ci
                            j = c - 4 * t
                            if 0 <= j < NB:
                                sl_ = ptile[:, ci, 128 * j:128 * j + 128]
                                nc.vector.tensor_mul(sl_, sl_, tri_sb[:])
                        cur.append((hl, c0, ptile))
                    # lagged attn@V: pairs lag two iterations behind the
                    # scores/exp stream, so these matmuls never wait on ACT
                    pending.append(cur)
                    if len(pending) > 2:
                        for args in pending.pop(0):
                            emit_attnv(*args)
                    drip(2 if t == 0 else 1)
                    if g == min(1, ncols // 2 - 1) and inject is not None:
                        inject()
                        inject = None
                if inject is not None:
                    inject()
                return lambda: attn_tail(t, hp, pacc, pending, emit_attnv,
                                         emit_attnv_chunk)

            def norm_cols(t, hp, pacc, lo, hi):
                # normalize attT columns [lo, hi): phase-split so the two
                # heads' recip -> broadcast -> mul chains pipeline across
                # DVE / Pool / DVE
                q0 = SQ * t
                recs, rbs = {}, {}
                for hl in range(2):
                    recs[hl] = rc_pool.tile([1, 512], F32, tag="rc",
                                            name="rec")
                    nc.vector.reciprocal(recs[hl][:, lo:hi],
                                         pacc[hl][64:65, lo:hi])
                for hl in range(2):
                    rbs[hl] = rb_pool.tile([64, 512], F32, tag="rb",
                                           name="rb")
                    nc.gpsimd.partition_broadcast(rbs[hl][:, lo:hi],
                                                  recs[hl][0:1, lo:hi],
                                                  channels=64)
                for hl in range(2):
                    nc.vector.tensor_mul(
                        attT[hp][64 * hl:64 * hl + 64, q0 + lo:q0 + hi],
                        pacc[hl][0:64, lo:hi], rbs[hl][0:64, lo:hi])

            def attn_tail(t, hp, pacc, pending, emit_attnv, emit_attnv_chunk):
                if (t, hp) != (NSL - 1, 1):
                    for cur in pending:
                        for args in cur:
                            emit_attnv(*args)
                    norm_cols(t, hp, pacc, 0, 512)
                    return
                # final pair: the last 4 chunks are the diagonal band, so
                # pacc column-quarter q is complete once chunk 4t+q has
                # accumulated — normalize progressively so the tail oproj
                # starts ~4us earlier
                for cur in pending[:-2]:
                    for args in cur:
                        emit_attnv(*args)
                for cur in pending[-2:]:
                    for ci in range(2):
                        for (hl, c0, ptile) in cur:
                            emit_attnv_chunk(hl, c0, ptile, ci)
                        qq = cur[0][1] + ci - 4 * t
                        norm_cols(t, hp, pacc, 128 * qq, 128 * qq + 128)

            def queue_qk(t):
                for w8_sb, dst8 in ((wq8_sb, q8), (wk8_sb, k8)):
                    for ab in range(2):
                        bg.append(lambda t=t, dst8=dst8, w8_sb=w8_sb, ab=ab:
                                  qk_chain(t, dst8, w8_sb, ab))

            def queue_v(t):
                for sc in range(4 * t, 4 * t + 4):
                    bg.append(lambda t=t, sc=sc: v_chain(t, sc))

            # ---- main slice-pipelined schedule -------------------------
            # qk chains of slice t+1 drain during attention(t); v chains of
            # slice t+1 drain at the START of attention(t+1) (their chunks
            # are only read from pair 2(t+1)); oproj units drip into any
            # window with PE slack (every window is ACT-bound in v3)
            queue_qk(0)
            queue_v(0)
            drip(4)                  # slice 0 q/k chains up front; v chains
            tail = None              # drip inside attention(0)
            for t in range(NSL):
                if t + 1 < NSL:
                    queue_qk(t + 1)
                bgo_budget[0] = (10 * EC if t == NSL - 1 else
                                 10 if t == NSL - 2 else 8)

                og_slice[t] = og_pool.tile([128, EC, 512], BF16, tag="og",
                                           name=f"og{t}")
                for hp in range(2):
                    tail = attn_head(t, hp, tail)
                    drip(1)
                    flush_og()
                if t + 1 < NSL:
                    queue_v(t + 1)
                # slice t attention done -> queue its output projection
                # as low-priority filler; slice 3's units run right here
                if t == NSL - 1:
                    tail()           # final pair's attn@V flush + norm
                    tail = None
                    flush_og()
                    bgo_budget[0] = 10 * EC
                    while bg or bgo:
                        drip(1)
                    flush_og()
                    stx = [st_pool.tile([128, 2, 512], F32, tag="st",
                                        name="stx") for _ in range(2)]
                    # tail oproj: column quarters so each matmul starts as
                    # soon as its progressive quarter-norm lands; copies
                    # split ACT (low half) / DVE (high half)
                    for et in range(EC):
                        if 2 <= et < 6:
                            ps = stx[(et - 2) // 2][:, et % 2, :]
                        else:
                            ps = gm_pool.tile([128, 512], F32, tag="gm",
                                              name="gm")[:]
                        o = SQ * t
                        for qq in range(4):
                            c0, c1 = 128 * qq, 128 * qq + 128
                            for hp_ in range(2):
                                nc.tensor.matmul(
                                    ps[:, c0:c1],
                                    wo_sb[:, hp_, 128 * et:128 * et + 128],
                                    attT[hp_][:, o + c0:o + c1],
                                    start=(hp_ == 0), stop=(hp_ == 1),
                                    skip_group_check=True)
                        nc.scalar.copy(og_slice[t][:, et, 0:256],
                                       ps[:, 0:256])
                        nc.vector.tensor_copy(og_slice[t][:, et, 256:512],
                                              ps[:, 256:512])
                        if et % 2 == 1:
                            nc.sync.dma_start(
                                oTr[:, et - 1:et + 1, o:o + 512],
                                og_slice[t][:, et - 1:et + 1, :])
                else:
                    for et in range(EC):
                        bgo.append(lambda t=t, et=et: oproj_unit(t, et))
                    flush_og()

            if debug:
                nc.sync.dma_start(dbg["d_q8"][:, :, :], q8[:])
                nc.sync.dma_start(dbg["d_k8"][:, :, :], k8[:])
                nc.sync.dma_start(
                    dbg["d_v"][:, :],
                    v_sb[:].rearrange("p a b c -> p (a b c)"))
                nc.sync.dma_start(dbg["d_attT0"][:, :], attT[0][:])
                nc.sync.dma_start(dbg["d_attT1"][:, :], attT[1][:])

    nc.compile()
    return nc


def permute_qk_cols(w):
    """[E, 256] -> chain A columns = (h, k) d=k, chain B -> d=32+k."""
    wg = np.asarray(w).reshape(E, HPC, D)
    a = wg[:, :, 0:32].reshape(E, 128)
    b = wg[:, :, 32:64].reshape(E, 128)
    return np.concatenate([a, b], axis=1)


def dr_weight_layout(wperm):
    """[E, 256] permuted -> [128, 2(ab), 4(b), 2(t), 128] fp8 with
    e = 256b + 128t + i."""
    w = np.asarray(wperm, dtype=np.float32) * WSCALE
    w = w.reshape(4, 2, 128, 2, 128)          # (b, t, i, ab, c)
    w = w.transpose(2, 3, 0, 1, 4)            # (i, ab, b, t, c)
    return np.ascontiguousarray(w).astype(NF8)


def dr_x_layout(xT):
    """x[b]^T [E, S] -> [128, 4(b), 2(t), S] fp8 with e = 256b + 128t + i."""
    x = np.asarray(xT, dtype=np.float32).reshape(4, 2, 128, S)
    x = x.transpose(2, 0, 1, 3)
    return np.ascontiguousarray(x).astype(NF8)


_NC_CACHE = None


def kernel(x, w_q, w_k, w_v, w_o):
    global _NC_CACHE
    if _NC_CACHE is None:
        _NC_CACHE = build_kernel()
    nc = _NC_CACHE

    x = np.asarray(x, dtype=np.float32)
    w_q = np.asarray(w_q, dtype=np.float32)
    w_k = np.asarray(w_k, dtype=np.float32)
    w_v = np.asarray(w_v, dtype=np.float32)
    w_o = np.asarray(w_o, dtype=np.float32)

    tri = make_tri()
    in_maps = []
    for core in range(NCORES):
        b, g = divmod(core, NCORES // B)
        sl = slice(g * DP, (g + 1) * DP)
        xT = np.ascontiguousarray(x[b].T)
        in_maps.append({
            "xT": xT.astype(NBF),
            "x8": dr_x_layout(xT),
            "wq8": dr_weight_layout(permute_qk_cols(w_q[:, sl])),
            "wk8": dr_weight_layout(permute_qk_cols(w_k[:, sl])),
            "wv": np.ascontiguousarray(w_v[:, sl]).astype(NBF),
            "wo": np.ascontiguousarray(w_o[sl, :]).astype(NBF),
            "tri": tri,
        })

    res = bass_utils.run_bass_kernel_spmd(nc, in_maps, core_ids=list(range(NCORES)))

    out = np.zeros((B, S, E), dtype=np.float32)
    for core in range(NCORES):
        b = core // (NCORES // B)
        out[b] += res.results[core]["oT"].astype(np.float32).T
    return out
